# revision 13
# baseline (speedup 1.0000x reference)
"""nn_CausalSelfAttention_88854283420050 — Bass/Tile kernel for 8 trn2 cores.

Sharding: tensor-parallel over heads (H=16 -> 2 heads per core).
Each core computes, for its 2 heads: the qkv projection (columns of
c_attn), per-head LayerNorm + RoPE, causal attention, and a partial
output projection y_c = O_heads @ W_proj[:, head cols].T.  The host
sums the 8 partial projections (row-parallel c_proj) and adds b_proj.

v2 design (vs v1 baseline):
  - bf16 data plane everywhere (fp32 accumulation in PSUM): halves DMA
    and SBUF traffic, enables FWL weight loads and DMA-engine transposes.
  - Causal mask folded into the S^T PSUM accumulation as an additive
    identity-matmul against a -1e5 step table (DVE mask mults gone).
  - q/k transposes to [c, t] layout done by the DMA xbar-transpose
    engine instead of PE+ACT.
  - LN stats via one DVE bn_stats + bn_aggr; LN apply on ACT via
    Identity activation with per-partition scale/bias APs.
  - 1/L broadcast via gpsimd partition_broadcast (frees a PSUM bank and
    the ACT copies of the old rank-1-matmul broadcast).
  - Weight DMA split per-contraction-chunk so the first QKV matmuls
    start ~2us in instead of ~30us.
  - Phase C (output projection + DMA out) software-pipelined into the
    attention loop (tb-outer), keeping PE dense and HAM warm.
"""
import math
import os
import sys

sys.path.insert(0, "/opt/trn_rl_repo")

import numpy as np
import ml_dtypes
from concourse import bacc, mybir, tile
from concourse import bass_utils

T, D, H, C = 2048, 2048, 16, 128
EPS = 1e-6
NCORES = 8
HPC = H // NCORES  # heads per core
DT = 17            # contraction tiles incl. bias row
F32 = mybir.dt.float32
BF16 = mybir.dt.bfloat16
AF = mybir.ActivationFunctionType
ALU = mybir.AluOpType
AX = mybir.AxisListType

NT = T // 128
NB = T // 512
NEG = -1.0e5

_NC_CACHE = None
LAST_RESULT = None


def _build_program():
    nc = bacc.Bacc("TRN2", target_bir_lowering=False, debug=False,
                   enable_asserts=True, num_devices=NCORES)

    xT = nc.dram_tensor("xT", [128, NT, DT, 128], BF16,
                    kind="ExternalInput").ap()
    wqkv = nc.dram_tensor("wqkv", [DT * 128, 6 * C], BF16,
                          kind="ExternalInput").ap()
    ropecos = nc.dram_tensor("ropecos", [T, 4 * C], BF16,
                             kind="ExternalInput").ap()
    ropesin = nc.dram_tensor("ropesin", [T, 4 * C], BF16,
                             kind="ExternalInput").ap()
    masks = nc.dram_tensor("masks", [128, 4 * 512], BF16,
                           kind="ExternalInput").ap()
    wp = nc.dram_tensor("wp", [HPC * C, D], BF16, kind="ExternalInput").ap()
    onescol = nc.dram_tensor("onescol", [128, 1], BF16,
                             kind="ExternalInput").ap()
    ident = nc.dram_tensor("ident", [128, 128], BF16,
                           kind="ExternalInput").ap()
    y = nc.dram_tensor("y", [T, D], BF16, kind="ExternalOutput").ap()

    sc = 1.0 / math.sqrt(C)

    with tile.TileContext(nc) as tc:
        with tc.tile_pool(name="res", bufs=1) as res:
            qkT = res.tile([128, 4, T], BF16, tag="qkT")  # [c, qh0|qh1|kh0|kh1, t]
            vv = res.tile([128, NT, HPC * C], BF16, tag="vv")  # [s, stile, hc]
            ot = res.tile([128, HPC, T], BF16, tag="ot")       # [c, h, t]
            ones_c = res.tile([128, 1], BF16, tag="onescol")
            id_sb = res.tile([128, 128], BF16, tag="ident")
            masks_sb = res.tile([128, 4 * 512], BF16, tag="masks")
            wp_sb = res.tile([128, HPC, D], BF16, tag="wp")

            zeros_c = res.tile([128, 1], F32, tag="zeros_c")
            eps_c = res.tile([128, 1], F32, tag="eps_c")
            nc.gpsimd.memset(zeros_c[:], 0.0)
            nc.gpsimd.memset(eps_c[:], EPS)

            # =========== Phase A: QKV projection + LN + RoPE ===========
            with (
                tc.tile_pool(name="wq", bufs=1) as wqp,
                tc.tile_pool(name="xcol", bufs=4) as xcolp,
                tc.tile_pool(name="qn", bufs=5) as qnp,
                tc.tile_pool(name="psA", bufs=4, space="PSUM") as psAp,
                tc.tile_pool(name="psB", bufs=3, space="PSUM") as psBp,
                tc.tile_pool(name="rope", bufs=4) as ropep,
                tc.tile_pool(name="lnst", bufs=2) as lnstp,
            ):
                w_sb = wqp.tile([128, DT, 6 * C], BF16, tag="w_sb")

                # Preamble: all input-DMA writes emitted before any reader
                # (program order drives Tile deps), with the first tiles'
                # x/rope loads interleaved ahead of most weight chunks in
                # sync-queue order so the first matmuls start ~2.5us in.
                pre = {}
                for pt in range(3):
                    xcol = xcolp.tile([128, DT, 128], BF16, tag="xcol")
                    nc.sync.dma_start(xcol[:], xT[:, pt, :, :])
                    rc = ropep.tile([128, 512], BF16, tag="ropec")
                    rs = ropep.tile([128, 512], BF16, tag="ropes")
                    nc.sync.dma_start(
                        rc[:], ropecos[pt * 128:(pt + 1) * 128, :])
                    nc.sync.dma_start(
                        rs[:], ropesin[pt * 128:(pt + 1) * 128, :])
                    pre[pt] = (xcol, rc, rs)
                    lo = [0, 6, 12][pt]
                    hi = [6, 12, DT][pt]
                    for dt in range(lo, hi):
                        nc.sync.dma_start(
                            w_sb[:, dt, :], wqkv[dt * 128:(dt + 1) * 128, :])

                qn_tiles = []
                for tt in range(NT):
                    if tt < 3:
                        xcol, rc, rs = pre[tt]
                    else:
                        xcol = xcolp.tile([128, DT, 128], BF16, tag="xcol")
                        nc.sync.dma_start(xcol[:], xT[:, tt, :, :])
                        rc = ropep.tile([128, 512], BF16, tag="ropec")
                        rs = ropep.tile([128, 512], BF16, tag="ropes")
                        nc.sync.dma_start(
                            rc[:], ropecos[tt * 128:(tt + 1) * 128, :])
                        nc.sync.dma_start(
                            rs[:], ropesin[tt * 128:(tt + 1) * 128, :])

                    psA = psAp.tile([128, 512], F32, tag="psA")
                    psB = psBp.tile([128, 256], F32, tag="psB")
                    for dt in range(DT):
                        nc.tensor.matmul(
                            psA[:], xcol[:, dt, :], w_sb[:, dt, 0:512],
                            start=(dt == 0), stop=(dt == DT - 1))
                        nc.tensor.matmul(
                            psB[:], xcol[:, dt, :], w_sb[:, dt, 512:768],
                            start=(dt == 0), stop=(dt == DT - 1))
                    # v: straight copy to SBUF in natural [s, c] layout
                    nc.scalar.activation(vv[:, tt, :], psB[:], AF.Copy)

                    # LN stats: sums on DVE, sum-squares via ACT Square
                    # accumulators, rstd in one ACT Abs_reciprocal_sqrt
                    st = lnstp.tile([128, 16], F32, tag="st")
                    # st cols: 0:4 sums, 4:8 sumsq, 8:12 mu, 12:16 musq
                    nc.vector.reduce_sum(
                        st[:, 0:4],
                        psA[:].rearrange("p (a b) -> p a b", a=4),
                        axis=AX.X)
                    for i in range(4):
                        sq = lnstp.tile([128, 128], F32, tag="sq")
                        nc.scalar.activation(
                            sq[:], psA[:, i * 128:(i + 1) * 128], AF.Square,
                            bias=zeros_c[:], accum_out=st[:, 4 + i:5 + i])
                    mu8 = lnstp.tile([128, 8], F32, tag="mu8")
                    var = lnstp.tile([128, 4], F32, tag="var")
                    rstd = lnstp.tile([128, 4], F32, tag="rstd")
                    nmu = lnstp.tile([128, 4], F32, tag="nmu")
                    nc.vector.tensor_scalar(
                        mu8[:], st[:, 0:8], 1.0 / C, None, op0=ALU.mult)
                    nc.vector.tensor_tensor(
                        var[:], mu8[:, 0:4], mu8[:, 0:4], op=ALU.mult)
                    nc.vector.tensor_tensor(
                        var[:], mu8[:, 4:8], var[:], op=ALU.subtract)
                    nc.scalar.activation(rstd[:], var[:],
                                         AF.Abs_reciprocal_sqrt,
                                         bias=eps_c[:])
                    nc.vector.scalar_tensor_tensor(
                        nmu[:], mu8[:, 0:4], -1.0, rstd[:],
                        op0=ALU.mult, op1=ALU.mult)

                    qn = qnp.tile([128, 512], BF16, tag="qn")
                    for i in range(4):
                        nc.vector.tensor_scalar(
                            qn[:, i * 128:(i + 1) * 128],
                            psA[:, i * 128:(i + 1) * 128],
                            rstd[:, i:i + 1], nmu[:, i:i + 1],
                            op0=ALU.mult, op1=ALU.add)

                    # RoPE (ln weights folded into the tables host-side)
                    rot = qnp.tile([128, 512], BF16, tag="rot")
                    qn3 = qn[:].rearrange("p (a b) -> p a b", b=2)
                    rot3 = rot[:].rearrange("p (a b) -> p a b", b=2)
                    nc.vector.tensor_scalar(
                        rot3[:, :, 0], qn3[:, :, 1], -1.0, None, op0=ALU.mult)
                    nc.vector.tensor_copy(rot3[:, :, 1], qn3[:, :, 0])
                    nc.vector.tensor_tensor(qn[:], qn[:], rc[:], op=ALU.mult)
                    nc.vector.tensor_tensor(rot[:], rot[:], rs[:],
                                            op=ALU.mult)
                    nc.vector.tensor_tensor(qn[:], qn[:], rot[:], op=ALU.add)

                    # q/k -> [c, t] layout via DMA xbar transpose (sync
                    # ring, lag-2: by the time it reaches the queue head its
                    # RoPE input is long done, so it never head-of-line
                    # blocks the input DMA stream)
                    qn_tiles.append((tt, qn))
                    if len(qn_tiles) > 2:
                        ptt, pqn = qn_tiles.pop(0)
                        nc.sync.dma_start_transpose(
                            qkT[:, :, ptt * 128:(ptt + 1) * 128], pqn[:])
                    if tt == 1:
                        nc.sync.dma_start(ones_c[:], onescol[:])
                        nc.sync.dma_start(id_sb[:], ident[:])
                        nc.sync.dma_start(masks_sb[:], masks[:])
                        nc.sync.dma_start(
                            wp_sb[:], wp.rearrange("(h p) d -> p h d", p=128))
                while qn_tiles:
                    ptt, pqn = qn_tiles.pop(0)
                    nc.sync.dma_start_transpose(
                        qkT[:, :, ptt * 128:(ptt + 1) * 128], pqn[:])

            # ====== Phase B+C: attention, fused with output projection ======
            with (
                tc.tile_pool(name="psS", bufs=3, space="PSUM") as psSp,
                tc.tile_pool(name="psL", bufs=1, space="PSUM") as psLp,
                tc.tile_pool(name="psO", bufs=2, space="PSUM") as psOp,
                tc.tile_pool(name="psY", bufs=2, space="PSUM") as psYp,
                tc.tile_pool(name="aT", bufs=3) as aTp,
                tc.tile_pool(name="bsm", bufs=2) as bsmp,
                tc.tile_pool(name="ysb", bufs=3) as ysbp,
            ):
                def emit_c(tb):
                    for ttt in range(4 * tb, 4 * (tb + 1)):
                        for db in range(NB):
                            yps = psYp.tile([128, 512], F32, tag="psY")
                            for h in range(HPC):
                                nc.tensor.matmul(
                                    yps[:],
                                    ot[:, h, ttt * 128:(ttt + 1) * 128],
                                    wp_sb[:, h, db * 512:(db + 1) * 512],
                                    start=(h == 0), stop=(h == HPC - 1))
                            ysb = ysbp.tile([128, 512], BF16, tag="ysb")
                            if db % 2 == 0:
                                nc.scalar.activation(ysb[:], yps[:], AF.Copy)
                            else:
                                nc.vector.tensor_copy(ysb[:], yps[:])
                            nc.sync.dma_start(
                                y[ttt * 128:(ttt + 1) * 128,
                                  db * 512:(db + 1) * 512],
                                ysb[:])

                for tb in range(NB):
                    for h in range(HPC):
                        S = 4 * (tb + 1)
                        qTs = qkT[:, h, tb * 512:(tb + 1) * 512]
                        st_ps = [None] * S

                        def emit_st(s):
                            stp = psSp.tile([128, 512], F32, tag="psS")
                            diag = s >= 4 * tb
                            nc.tensor.matmul(
                                stp[:], qkT[:, 2 + h, s * 128:(s + 1) * 128],
                                qTs, start=True, stop=not diag)
                            if diag:
                                j = s - 4 * tb
                                nc.tensor.matmul(
                                    stp[:], id_sb[:],
                                    masks_sb[:, j * 512:(j + 1) * 512],
                                    start=False, stop=True)
                            st_ps[s] = stp

                        Lps = psLp.tile([1, 512], F32, tag="psL")
                        Ops = psOp.tile([128, 512], F32, tag="psO")
                        emit_st(0)
                        if S > 1:
                            emit_st(1)
                        for s in range(S):
                            if s + 2 < S:
                                emit_st(s + 2)
                            a = aTp.tile([128, 512], BF16, tag="aT")
                            nc.scalar.activation(
                                a[:], st_ps[s][:], AF.Exp,
                                bias=zeros_c[:], scale=sc)
                            st_ps[s] = None
                            nc.tensor.matmul(
                                Lps[:], ones_c[:], a[:],
                                start=(s == 0), stop=(s == S - 1))
                            nc.tensor.matmul(
                                Ops[:], vv[:, s, h * C:(h + 1) * C], a[:],
                                start=(s == 0), stop=(s == S - 1))
                        # 1/L = exp(-ln(L)) on ACT: reads PSUM directly
                        # (frees the L bank fast) and keeps the DVE queue
                        # out of the normalization chain
                        lnL = bsmp.tile([1, 512], F32, tag="lnL")
                        nc.scalar.activation(lnL[:], Lps[:], AF.Ln,
                                             bias=zeros_c[0:1, :])
                        recL = bsmp.tile([1, 512], F32, tag="recL")
                        nc.scalar.activation(recL[:], lnL[:], AF.Exp,
                                             bias=zeros_c[0:1, :], scale=-1.0)
                        bcs = bsmp.tile([128, 512], F32, tag="bcs")
                        nc.gpsimd.partition_broadcast(bcs[:], recL[:])
                        nc.vector.tensor_tensor(
                            ot[:, h, tb * 512:(tb + 1) * 512], Ops[:],
                            bcs[:], op=ALU.mult)
                    if tb >= 1:
                        emit_c(tb - 1)
                emit_c(NB - 1)

    nc.compile()
    return nc


def _host_prep(x, W_attn, b_attn, W_proj, q_ln_w, k_ln_w):
    f = np.float32
    bf = ml_dtypes.bfloat16
    xTf = np.zeros((DT * 128, T), f)
    xTf[:D] = x.T
    xTf[D] = 1.0
    # [p, tt, dt, j] = xTf[dt*128 + p, tt*128 + j]: one contiguous run per
    # (partition, tile) for full-bandwidth DMA descriptors
    xT = np.ascontiguousarray(
        xTf.reshape(DT, 128, NT, 128).transpose(1, 2, 0, 3)).astype(bf)

    inv = (1.0 / (10000.0 ** (np.arange(0, C, 2, dtype=f) / C))).astype(f)
    freqs = np.arange(T, dtype=f)[:, None] * inv[None, :]
    sin = np.repeat(np.sin(freqs), 2, axis=1).astype(f)
    cos = np.repeat(np.cos(freqs), 2, axis=1).astype(f)
    part = np.arange(C) ^ 1
    cos_q = cos * q_ln_w[None, :]
    sin_q = sin * q_ln_w[None, part]
    cos_k = cos * k_ln_w[None, :]
    sin_k = sin * k_ln_w[None, part]
    ropecos = np.ascontiguousarray(
        np.concatenate([cos_q, cos_q, cos_k, cos_k], axis=1).astype(bf))
    ropesin = np.ascontiguousarray(
        np.concatenate([sin_q, sin_q, sin_k, sin_k], axis=1).astype(bf))

    ss = np.arange(128)[:, None]
    ttm = np.arange(512)[None, :]
    masks = np.ascontiguousarray(np.concatenate(
        [(j * 128 + ss > ttm).astype(f) * NEG for j in range(4)],
        axis=1).astype(bf))

    shared = dict(xT=xT, ropecos=ropecos, ropesin=ropesin, masks=masks,
                  onescol=np.ones((128, 1), bf),
                  ident=np.eye(128, dtype=bf))

    in_maps = []
    for c in range(NCORES):
        h0, h1 = HPC * c, HPC * c + 1
        rows = np.concatenate([
            np.arange(h0 * C, (h0 + 1) * C),
            np.arange(h1 * C, (h1 + 1) * C),
            D + np.arange(h0 * C, (h0 + 1) * C),
            D + np.arange(h1 * C, (h1 + 1) * C),
            2 * D + np.arange(h0 * C, (h0 + 1) * C),
            2 * D + np.arange(h1 * C, (h1 + 1) * C),
        ])
        wqkv = np.zeros((DT * 128, 6 * C), f)
        wqkv[:D] = W_attn[rows].T
        wqkv[D] = b_attn[rows]
        wpc = np.concatenate(
            [W_proj[:, h0 * C:(h0 + 1) * C].T,
             W_proj[:, h1 * C:(h1 + 1) * C].T], axis=0)
        m = dict(shared)
        m["wqkv"] = np.ascontiguousarray(wqkv.astype(bf))
        m["wp"] = np.ascontiguousarray(wpc.astype(bf))
        in_maps.append(m)
    return in_maps


def kernel(x, W_attn, b_attn, W_proj, b_proj, q_ln_w, k_ln_w):
    global _NC_CACHE, LAST_RESULT
    f = np.float32
    x = np.ascontiguousarray(np.asarray(x, f))
    W_attn = np.ascontiguousarray(np.asarray(W_attn, f))
    b_attn = np.ascontiguousarray(np.asarray(b_attn, f))
    W_proj = np.ascontiguousarray(np.asarray(W_proj, f))
    b_proj = np.ascontiguousarray(np.asarray(b_proj, f))
    q_ln_w = np.ascontiguousarray(np.asarray(q_ln_w, f))
    k_ln_w = np.ascontiguousarray(np.asarray(k_ln_w, f))

    in_maps = _host_prep(x, W_attn, b_attn, W_proj, q_ln_w, k_ln_w)
    if _NC_CACHE is None:
        _NC_CACHE = _build_program()
    nc = _NC_CACHE

    res = bass_utils.run_bass_kernel_spmd(
        nc, in_maps, core_ids=list(range(NCORES)),
        trace=bool(os.environ.get("BASS_TRACE")))
    LAST_RESULT = res

    y = np.zeros((T, D), np.float32)
    for rmap in res.results:
        y += rmap["y"].astype(np.float32)
    y += b_proj[None, :]
    return y


# revision 14
# speedup vs baseline: 1.0303x; 1.0303x over previous
"""nn_CausalSelfAttention_88854283420050 — Bass/Tile kernel for 8 trn2 cores.

Sharding: tensor-parallel over heads (H=16 -> 2 heads per core).
Each core computes, for its 2 heads: the qkv projection (columns of
c_attn), per-head LayerNorm + RoPE, causal attention, and a partial
output projection y_c = O_heads @ W_proj[:, head cols].T.  The host
sums the 8 partial projections (row-parallel c_proj) and adds b_proj.

v2 design (vs v1 baseline):
  - bf16 data plane everywhere (fp32 accumulation in PSUM): halves DMA
    and SBUF traffic, enables FWL weight loads and DMA-engine transposes.
  - Causal mask folded into the S^T PSUM accumulation as an additive
    identity-matmul against a -1e5 step table (DVE mask mults gone).
  - q/k transposes to [c, t] layout done by the DMA xbar-transpose
    engine instead of PE+ACT.
  - LN stats via one DVE bn_stats + bn_aggr; LN apply on ACT via
    Identity activation with per-partition scale/bias APs.
  - 1/L broadcast via gpsimd partition_broadcast (frees a PSUM bank and
    the ACT copies of the old rank-1-matmul broadcast).
  - Weight DMA split per-contraction-chunk so the first QKV matmuls
    start ~2us in instead of ~30us.
  - Phase C (output projection + DMA out) software-pipelined into the
    attention loop (tb-outer), keeping PE dense and HAM warm.
"""
import math
import os
import sys

sys.path.insert(0, "/opt/trn_rl_repo")

import numpy as np
import ml_dtypes
from concourse import bacc, mybir, tile
from concourse import bass_utils

T, D, H, C = 2048, 2048, 16, 128
EPS = 1e-6
NCORES = 8
HPC = H // NCORES  # heads per core
DT = 17            # contraction tiles incl. bias row
F32 = mybir.dt.float32
BF16 = mybir.dt.bfloat16
AF = mybir.ActivationFunctionType
ALU = mybir.AluOpType
AX = mybir.AxisListType

NT = T // 128
NB = T // 512
NEG = -1.0e5

_NC_CACHE = None
LAST_RESULT = None


def _build_program():
    nc = bacc.Bacc("TRN2", target_bir_lowering=False, debug=False,
                   enable_asserts=True, num_devices=NCORES)

    xT = nc.dram_tensor("xT", [128, NT, DT, 128], BF16,
                    kind="ExternalInput").ap()
    wqkv = nc.dram_tensor("wqkv", [DT * 128, 6 * C], BF16,
                          kind="ExternalInput").ap()
    ropecos = nc.dram_tensor("ropecos", [T, 4 * C], BF16,
                             kind="ExternalInput").ap()
    ropesin = nc.dram_tensor("ropesin", [T, 4 * C], BF16,
                             kind="ExternalInput").ap()
    masks = nc.dram_tensor("masks", [128, 4 * 512], BF16,
                           kind="ExternalInput").ap()
    wp = nc.dram_tensor("wp", [HPC * C, D], BF16, kind="ExternalInput").ap()
    onescol = nc.dram_tensor("onescol", [128, 1], BF16,
                             kind="ExternalInput").ap()
    ident = nc.dram_tensor("ident", [128, 128], BF16,
                           kind="ExternalInput").ap()
    y = nc.dram_tensor("y", [T, D], BF16, kind="ExternalOutput").ap()

    sc = 1.0 / math.sqrt(C)

    with tile.TileContext(nc) as tc:
        with tc.tile_pool(name="res", bufs=1) as res:
            qkT = res.tile([128, 4, T], BF16, tag="qkT")  # [c, qh0|qh1|kh0|kh1, t]
            vv = res.tile([128, NT, HPC * C], BF16, tag="vv")  # [s, stile, hc]
            ot = res.tile([128, HPC, T], BF16, tag="ot")       # [c, h, t]
            ones_c = res.tile([128, 1], BF16, tag="onescol")
            id_sb = res.tile([128, 128], BF16, tag="ident")
            masks_sb = res.tile([128, 4 * 512], BF16, tag="masks")
            wp_sb = res.tile([128, HPC, D], BF16, tag="wp")

            zeros_c = res.tile([128, 1], F32, tag="zeros_c")
            eps_c = res.tile([128, 1], F32, tag="eps_c")
            nc.gpsimd.memset(zeros_c[:], 0.0)
            nc.gpsimd.memset(eps_c[:], EPS)

            # =========== Phase A: QKV projection + LN + RoPE ===========
            with (
                tc.tile_pool(name="wq", bufs=1) as wqp,
                tc.tile_pool(name="xcol", bufs=4) as xcolp,
                tc.tile_pool(name="qn", bufs=5) as qnp,
                tc.tile_pool(name="psA", bufs=4, space="PSUM") as psAp,
                tc.tile_pool(name="psB", bufs=3, space="PSUM") as psBp,
                tc.tile_pool(name="rope", bufs=4) as ropep,
                tc.tile_pool(name="lnst", bufs=2) as lnstp,
            ):
                w_sb = wqp.tile([128, DT, 6 * C], BF16, tag="w_sb")

                # Preamble: all input-DMA writes emitted before any reader
                # (program order drives Tile deps), with the first tiles'
                # x/rope loads interleaved ahead of most weight chunks in
                # sync-queue order so the first matmuls start ~2.5us in.
                pre = {}
                for pt in range(3):
                    xcol = xcolp.tile([128, DT, 128], BF16, tag="xcol")
                    nc.sync.dma_start(xcol[:], xT[:, pt, :, :])
                    rc = ropep.tile([128, 512], BF16, tag="ropec")
                    rs = ropep.tile([128, 512], BF16, tag="ropes")
                    nc.sync.dma_start(
                        rc[:], ropecos[pt * 128:(pt + 1) * 128, :])
                    nc.sync.dma_start(
                        rs[:], ropesin[pt * 128:(pt + 1) * 128, :])
                    pre[pt] = (xcol, rc, rs)
                    lo = [0, 6, 12][pt]
                    hi = [6, 12, DT][pt]
                    for dt in range(lo, hi):
                        nc.sync.dma_start(
                            w_sb[:, dt, :], wqkv[dt * 128:(dt + 1) * 128, :])

                qn_tiles = []
                for tt in range(NT):
                    if tt < 3:
                        xcol, rc, rs = pre[tt]
                    else:
                        xcol = xcolp.tile([128, DT, 128], BF16, tag="xcol")
                        nc.sync.dma_start(xcol[:], xT[:, tt, :, :])
                        rc = ropep.tile([128, 512], BF16, tag="ropec")
                        rs = ropep.tile([128, 512], BF16, tag="ropes")
                        nc.sync.dma_start(
                            rc[:], ropecos[tt * 128:(tt + 1) * 128, :])
                        nc.sync.dma_start(
                            rs[:], ropesin[tt * 128:(tt + 1) * 128, :])

                    psA = psAp.tile([128, 512], F32, tag="psA")
                    psB = psBp.tile([128, 256], F32, tag="psB")
                    for dt in range(DT):
                        nc.tensor.matmul(
                            psA[:], xcol[:, dt, :], w_sb[:, dt, 0:512],
                            start=(dt == 0), stop=(dt == DT - 1))
                        nc.tensor.matmul(
                            psB[:], xcol[:, dt, :], w_sb[:, dt, 512:768],
                            start=(dt == 0), stop=(dt == DT - 1))
                    # v: straight copy to SBUF in natural [s, c] layout
                    nc.scalar.activation(vv[:, tt, :], psB[:], AF.Copy)

                    # LN stats: sums on DVE, sum-squares via ACT Square
                    # accumulators, rstd in one ACT Abs_reciprocal_sqrt
                    st = lnstp.tile([128, 16], F32, tag="st")
                    # st cols: 0:4 sums, 4:8 sumsq, 8:12 mu, 12:16 musq
                    nc.vector.reduce_sum(
                        st[:, 0:4],
                        psA[:].rearrange("p (a b) -> p a b", a=4),
                        axis=AX.X)
                    for i in range(4):
                        sq = lnstp.tile([128, 128], F32, tag="sq")
                        nc.scalar.activation(
                            sq[:], psA[:, i * 128:(i + 1) * 128], AF.Square,
                            bias=zeros_c[:], accum_out=st[:, 4 + i:5 + i])
                    mu8 = lnstp.tile([128, 8], F32, tag="mu8")
                    var = lnstp.tile([128, 4], F32, tag="var")
                    rstd = lnstp.tile([128, 4], F32, tag="rstd")
                    nmu = lnstp.tile([128, 4], F32, tag="nmu")
                    nc.vector.tensor_scalar(
                        mu8[:], st[:, 0:8], 1.0 / C, None, op0=ALU.mult)
                    nc.vector.tensor_tensor(
                        var[:], mu8[:, 0:4], mu8[:, 0:4], op=ALU.mult)
                    nc.vector.tensor_tensor(
                        var[:], mu8[:, 4:8], var[:], op=ALU.subtract)
                    nc.scalar.activation(rstd[:], var[:],
                                         AF.Abs_reciprocal_sqrt,
                                         bias=eps_c[:])
                    nc.vector.scalar_tensor_tensor(
                        nmu[:], mu8[:, 0:4], -1.0, rstd[:],
                        op0=ALU.mult, op1=ALU.mult)

                    qn = qnp.tile([128, 512], BF16, tag="qn")
                    for i in range(4):
                        nc.vector.tensor_scalar(
                            qn[:, i * 128:(i + 1) * 128],
                            psA[:, i * 128:(i + 1) * 128],
                            rstd[:, i:i + 1], nmu[:, i:i + 1],
                            op0=ALU.mult, op1=ALU.add)

                    # RoPE (ln weights folded into the tables host-side)
                    rot = qnp.tile([128, 512], BF16, tag="rot")
                    qn3 = qn[:].rearrange("p (a b) -> p a b", b=2)
                    rot3 = rot[:].rearrange("p (a b) -> p a b", b=2)
                    nc.scalar.activation(rot3[:, :, 0], qn3[:, :, 1],
                                         AF.Copy, scale=-1.0)
                    nc.scalar.activation(rot3[:, :, 1], qn3[:, :, 0], AF.Copy)
                    nc.vector.tensor_tensor(qn[:], qn[:], rc[:], op=ALU.mult)
                    nc.vector.tensor_tensor(rot[:], rot[:], rs[:],
                                            op=ALU.mult)
                    nc.vector.tensor_tensor(qn[:], qn[:], rot[:], op=ALU.add)

                    # q/k -> [c, t] layout via DMA xbar transpose (sync
                    # ring, lag-2: by the time it reaches the queue head its
                    # RoPE input is long done, so it never head-of-line
                    # blocks the input DMA stream)
                    qn_tiles.append((tt, qn))
                    if len(qn_tiles) > 2:
                        ptt, pqn = qn_tiles.pop(0)
                        nc.sync.dma_start_transpose(
                            qkT[:, :, ptt * 128:(ptt + 1) * 128], pqn[:])
                    if tt == 1:
                        nc.sync.dma_start(ones_c[:], onescol[:])
                        nc.sync.dma_start(id_sb[:], ident[:])
                        nc.sync.dma_start(masks_sb[:], masks[:])
                        nc.sync.dma_start(
                            wp_sb[:], wp.rearrange("(h p) d -> p h d", p=128))
                while qn_tiles:
                    ptt, pqn = qn_tiles.pop(0)
                    nc.sync.dma_start_transpose(
                        qkT[:, :, ptt * 128:(ptt + 1) * 128], pqn[:])

            # ====== Phase B+C: attention, fused with output projection ======
            with (
                tc.tile_pool(name="psS", bufs=3, space="PSUM") as psSp,
                tc.tile_pool(name="psL", bufs=1, space="PSUM") as psLp,
                tc.tile_pool(name="psO", bufs=2, space="PSUM") as psOp,
                tc.tile_pool(name="psY", bufs=2, space="PSUM") as psYp,
                tc.tile_pool(name="aT", bufs=3) as aTp,
                tc.tile_pool(name="bsm", bufs=2) as bsmp,
                tc.tile_pool(name="ysb", bufs=3) as ysbp,
            ):
                def emit_c(tb):
                    for ttt in range(4 * tb, 4 * (tb + 1)):
                        for db in range(NB):
                            yps = psYp.tile([128, 512], F32, tag="psY")
                            for h in range(HPC):
                                nc.tensor.matmul(
                                    yps[:],
                                    ot[:, h, ttt * 128:(ttt + 1) * 128],
                                    wp_sb[:, h, db * 512:(db + 1) * 512],
                                    start=(h == 0), stop=(h == HPC - 1))
                            ysb = ysbp.tile([128, 512], BF16, tag="ysb")
                            if db % 2 == 0:
                                nc.scalar.activation(ysb[:], yps[:], AF.Copy)
                            else:
                                nc.vector.tensor_copy(ysb[:], yps[:])
                            nc.sync.dma_start(
                                y[ttt * 128:(ttt + 1) * 128,
                                  db * 512:(db + 1) * 512],
                                ysb[:])

                for tb in range(NB):
                    for h in range(HPC):
                        S = 4 * (tb + 1)
                        qTs = qkT[:, h, tb * 512:(tb + 1) * 512]
                        st_ps = [None] * S

                        def emit_st(s):
                            stp = psSp.tile([128, 512], F32, tag="psS")
                            diag = s >= 4 * tb
                            nc.tensor.matmul(
                                stp[:], qkT[:, 2 + h, s * 128:(s + 1) * 128],
                                qTs, start=True, stop=not diag)
                            if diag:
                                j = s - 4 * tb
                                nc.tensor.matmul(
                                    stp[:], id_sb[:],
                                    masks_sb[:, j * 512:(j + 1) * 512],
                                    start=False, stop=True)
                            st_ps[s] = stp

                        Lps = psLp.tile([1, 512], F32, tag="psL")
                        Ops = psOp.tile([128, 512], F32, tag="psO")
                        emit_st(0)
                        if S > 1:
                            emit_st(1)
                        a_tiles = [None] * S
                        for s in range(S):
                            if s + 2 < S:
                                emit_st(s + 2)
                            a = aTp.tile([128, 512], BF16, tag="aT")
                            nc.scalar.activation(
                                a[:], st_ps[s][:], AF.Exp,
                                bias=zeros_c[:], scale=sc)
                            st_ps[s] = None
                            a_tiles[s] = a
                            # L-matmul lags one s-tile: when the group's
                            # first L hits the PE queue head, the previous
                            # group's Ln has had time to free the psL bank
                            if s >= 1:
                                nc.tensor.matmul(
                                    Lps[:], ones_c[:], a_tiles[s - 1][:],
                                    start=(s == 1), stop=False)
                            nc.tensor.matmul(
                                Ops[:], vv[:, s, h * C:(h + 1) * C], a[:],
                                start=(s == 0), stop=(s == S - 1))
                        nc.tensor.matmul(
                            Lps[:], ones_c[:], a_tiles[S - 1][:],
                            start=(S == 1), stop=True)
                        # 1/L = exp(-ln(L)) on ACT: reads PSUM directly
                        # (frees the L bank fast) and keeps the DVE queue
                        # out of the normalization chain
                        lnL = bsmp.tile([1, 512], F32, tag="lnL")
                        nc.scalar.activation(lnL[:], Lps[:], AF.Ln,
                                             bias=zeros_c[0:1, :])
                        recL = bsmp.tile([1, 512], F32, tag="recL")
                        nc.scalar.activation(recL[:], lnL[:], AF.Exp,
                                             bias=zeros_c[0:1, :], scale=-1.0)
                        bcs = bsmp.tile([128, 512], F32, tag="bcs")
                        nc.gpsimd.partition_broadcast(bcs[:], recL[:])
                        nc.vector.tensor_tensor(
                            ot[:, h, tb * 512:(tb + 1) * 512], Ops[:],
                            bcs[:], op=ALU.mult)
                    if tb >= 1:
                        emit_c(tb - 1)
                emit_c(NB - 1)

    nc.compile()
    return nc


def _host_prep(x, W_attn, b_attn, W_proj, q_ln_w, k_ln_w):
    f = np.float32
    bf = ml_dtypes.bfloat16
    xTf = np.zeros((DT * 128, T), f)
    xTf[:D] = x.T
    xTf[D] = 1.0
    # [p, tt, dt, j] = xTf[dt*128 + p, tt*128 + j]: one contiguous run per
    # (partition, tile) for full-bandwidth DMA descriptors
    xT = np.ascontiguousarray(
        xTf.reshape(DT, 128, NT, 128).transpose(1, 2, 0, 3)).astype(bf)

    inv = (1.0 / (10000.0 ** (np.arange(0, C, 2, dtype=f) / C))).astype(f)
    freqs = np.arange(T, dtype=f)[:, None] * inv[None, :]
    sin = np.repeat(np.sin(freqs), 2, axis=1).astype(f)
    cos = np.repeat(np.cos(freqs), 2, axis=1).astype(f)
    part = np.arange(C) ^ 1
    cos_q = cos * q_ln_w[None, :]
    sin_q = sin * q_ln_w[None, part]
    cos_k = cos * k_ln_w[None, :]
    sin_k = sin * k_ln_w[None, part]
    ropecos = np.ascontiguousarray(
        np.concatenate([cos_q, cos_q, cos_k, cos_k], axis=1).astype(bf))
    ropesin = np.ascontiguousarray(
        np.concatenate([sin_q, sin_q, sin_k, sin_k], axis=1).astype(bf))

    ss = np.arange(128)[:, None]
    ttm = np.arange(512)[None, :]
    masks = np.ascontiguousarray(np.concatenate(
        [(j * 128 + ss > ttm).astype(f) * NEG for j in range(4)],
        axis=1).astype(bf))

    shared = dict(xT=xT, ropecos=ropecos, ropesin=ropesin, masks=masks,
                  onescol=np.ones((128, 1), bf),
                  ident=np.eye(128, dtype=bf))

    in_maps = []
    for c in range(NCORES):
        h0, h1 = HPC * c, HPC * c + 1
        rows = np.concatenate([
            np.arange(h0 * C, (h0 + 1) * C),
            np.arange(h1 * C, (h1 + 1) * C),
            D + np.arange(h0 * C, (h0 + 1) * C),
            D + np.arange(h1 * C, (h1 + 1) * C),
            2 * D + np.arange(h0 * C, (h0 + 1) * C),
            2 * D + np.arange(h1 * C, (h1 + 1) * C),
        ])
        wqkv = np.zeros((DT * 128, 6 * C), f)
        wqkv[:D] = W_attn[rows].T
        wqkv[D] = b_attn[rows]
        wpc = np.concatenate(
            [W_proj[:, h0 * C:(h0 + 1) * C].T,
             W_proj[:, h1 * C:(h1 + 1) * C].T], axis=0)
        m = dict(shared)
        m["wqkv"] = np.ascontiguousarray(wqkv.astype(bf))
        m["wp"] = np.ascontiguousarray(wpc.astype(bf))
        in_maps.append(m)
    return in_maps


def kernel(x, W_attn, b_attn, W_proj, b_proj, q_ln_w, k_ln_w):
    global _NC_CACHE, LAST_RESULT
    f = np.float32
    x = np.ascontiguousarray(np.asarray(x, f))
    W_attn = np.ascontiguousarray(np.asarray(W_attn, f))
    b_attn = np.ascontiguousarray(np.asarray(b_attn, f))
    W_proj = np.ascontiguousarray(np.asarray(W_proj, f))
    b_proj = np.ascontiguousarray(np.asarray(b_proj, f))
    q_ln_w = np.ascontiguousarray(np.asarray(q_ln_w, f))
    k_ln_w = np.ascontiguousarray(np.asarray(k_ln_w, f))

    in_maps = _host_prep(x, W_attn, b_attn, W_proj, q_ln_w, k_ln_w)
    if _NC_CACHE is None:
        _NC_CACHE = _build_program()
    nc = _NC_CACHE

    res = bass_utils.run_bass_kernel_spmd(
        nc, in_maps, core_ids=list(range(NCORES)),
        trace=bool(os.environ.get("BASS_TRACE")))
    LAST_RESULT = res

    y = np.zeros((T, D), np.float32)
    for rmap in res.results:
        y += rmap["y"].astype(np.float32)
    y += b_proj[None, :]
    return y
